# revision 6
# baseline (speedup 1.0000x reference)
"""Trainium2 Bass kernel for DavisManifold.distance (arc length under learned
metric G(p) = L Lᵀ + eps·I, L = lower-tri from a 2-layer MLP).

Key algebra: inner = vᵀ G v = ‖Lᵀ v‖² + MIN_EIG·‖v‖², so G (B×64×64) is never
materialized.  z = Lᵀv per batch row: z_j = Σ_{i≥j} C_ij·v_i with the diagonal
of C replaced by softplus(C_jj)+0.1.  The lower triangle (2080 entries) is laid
out as three 32×32 blocks (TL / BR triangles + BL dense), zero-padded to 3072
columns of a host-permuted W2.  Then:
  - comp = h @ W2perm  (PE matmuls, batch rows on partitions)
  - u = comp ⊙ v[row(pos)]  (DVE tensor_tensor with overlapping strided views
    of v — no gather tensor is ever built)
  - z = fixed-count strided reductions over the block grids (one DVE reduce)
  - diag correction via a separate 64-column matmul + softplus
Sharding: pure data parallel, batch split across 8 cores, weights replicated.
"""

import math
import os
import numpy as np
from contextlib import ExitStack

import concourse.bass as bass
import concourse.bacc as bacc
import concourse.tile as tile
from concourse import mybir
from concourse.bass_utils import run_bass_kernel_spmd

F32 = mybir.dt.float32
AF = mybir.ActivationFunctionType
ALU = mybir.AluOpType

DIM = 64
HID = 128
N_COMP = DIM * (DIM + 1) // 2  # 2080
MIN_EIG = 0.01
B_FULL = 32768
N_CORES = 8
B_CORE = B_FULL // N_CORES  # 4096
P = 128                      # rows per tile (SBUF partitions)
N_TILES = B_CORE // P        # 32
GRID = 3072                  # 3 padded 32x32 blocks * 32 cols

# matmul dtype for the big W2 matmuls: float32 (exact, 4 cyc/row),
# float32r (1 cyc/row at N>=256, reduced precision) — tuned below.
MM_DT = mybir.dt.float32r if os.environ.get("KERNEL_MM_DT", "f32r") == "f32r" else F32


def _tri_idx(i, j):
    return i * (i + 1) // 2 + j


def _build_w2p(W2):
    """Permute/pad W2 [HID, 2080] -> [HID, 3072] grid layout.
    cols 0:1024    TL: col = 32*j + s       <-> (i=j+s, j),   j<32, j+s<=31
    cols 1024:2048 BR: col = 1024+32*m+s    <-> (i=j+s, j=32+m), s<=31-m
    cols 2048:3072 BL: col = 2048+32*j+(i-32) <-> (i, j), i>=32, j<32
    """
    W2p = np.zeros((HID, GRID), dtype=np.float32)
    for j in range(32):
        for s in range(32 - j):
            W2p[:, 32 * j + s] = W2[:, _tri_idx(j + s, j)]
    for m in range(32):
        j = 32 + m
        for s in range(32 - m):
            W2p[:, 1024 + 32 * m + s] = W2[:, _tri_idx(j + s, j)]
    for j in range(32):
        for i in range(32, 64):
            W2p[:, 2048 + 32 * j + (i - 32)] = W2[:, _tri_idx(i, j)]
    return W2p


def _build_program(n_steps, b2_nonzero):
    nc = bacc.Bacc("TRN2", target_bir_lowering=False, debug=False,
                   num_devices=N_CORES)

    xin = nc.dram_tensor("xin", [B_CORE, DIM], F32, kind="ExternalInput")
    yin = nc.dram_tensor("yin", [B_CORE, DIM], F32, kind="ExternalInput")
    w1aug = nc.dram_tensor("w1aug", [DIM + 1, HID], F32, kind="ExternalInput")
    w2p = nc.dram_tensor("w2p", [HID, GRID], F32, kind="ExternalInput")
    w2d = nc.dram_tensor("w2d", [HID, DIM], F32, kind="ExternalInput")
    ident = nc.dram_tensor("ident", [P, P], F32, kind="ExternalInput")
    cb = nc.dram_tensor("cb", [DIM, DIM], F32, kind="ExternalInput")
    b2d = nc.dram_tensor("b2d", [1, DIM], F32, kind="ExternalInput")
    outt = nc.dram_tensor("outt", [B_CORE, 1], F32, kind="ExternalOutput")

    inv_n = 1.0 / n_steps

    with tile.TileContext(nc) as tc, ExitStack() as ctx:
        consts = ctx.enter_context(tc.tile_pool(name="consts", bufs=1))
        io = ctx.enter_context(tc.tile_pool(name="io", bufs=3))
        work = ctx.enter_context(tc.tile_pool(name="work", bufs=2))
        upool = ctx.enter_context(tc.tile_pool(name="upool", bufs=2))
        small = ctx.enter_context(tc.tile_pool(name="small", bufs=3))
        ps_a = ctx.enter_context(tc.tile_pool(name="ps_a", bufs=2, space="PSUM"))
        ps_c = ctx.enter_context(tc.tile_pool(name="ps_c", bufs=3, space="PSUM"))
        ps_h = ctx.enter_context(tc.tile_pool(name="ps_h", bufs=2, space="PSUM"))
        ps_d = ctx.enter_context(tc.tile_pool(name="ps_d", bufs=1, space="PSUM"))

        w2p_f32 = consts.tile([HID, GRID], F32)
        nc.sync.dma_start(out=w2p_f32, in_=w2p.ap())
        if MM_DT == F32:
            w2p_sb = w2p_f32
        else:
            # explicit rounding pass: BIR verifier requires fp32r matmul
            # inputs to be produced as fp32r
            w2p_sb = consts.tile([HID, GRID], MM_DT)
            nc.vector.tensor_copy(w2p_sb, w2p_f32)
        w2d_sb = consts.tile([HID, DIM], F32)
        nc.sync.dma_start(out=w2d_sb, in_=w2d.ap())
        w1aug_sb = consts.tile([DIM + 1, HID], F32)
        nc.sync.dma_start(out=w1aug_sb, in_=w1aug.ap())
        ident_sb = consts.tile([P, P], F32)
        nc.sync.dma_start(out=ident_sb, in_=ident.ap())
        thr_sb = consts.tile([P, 1], F32)
        nc.vector.memset(thr_sb, 1e-8)
        if b2_nonzero:
            cb_sb = consts.tile([DIM, DIM], F32)
            nc.sync.dma_start(out=cb_sb, in_=cb.ap())
            b2d_sb = consts.tile([P, DIM], F32)
            nc.sync.dma_start(out=b2d_sb, in_=b2d.ap().partition_broadcast(P))

        inner_all = consts.tile([P, N_TILES, n_steps], F32)
        vsq_all = consts.tile([P, N_TILES], F32)

        def vview(v_ap, off, jstep):
            return bass.AP(tensor=v_ap.tensor, offset=v_ap.offset + off,
                           ap=[v_ap.ap[0], [jstep, 16], [1, 32]])

        for it in range(N_TILES):
            r0 = it * P
            x_t = io.tile([P, DIM], F32, tag="x")
            nc.sync.dma_start(out=x_t, in_=xin.ap()[r0:r0 + P, :])
            y_t = io.tile([P, DIM], F32, tag="y")
            nc.sync.dma_start(out=y_t, in_=yin.ap()[r0:r0 + P, :])

            diff = small.tile([P, DIM], F32, tag="diff")
            nc.vector.tensor_tensor(diff, y_t, x_t, ALU.subtract)
            v_t = small.tile([P, 96], F32, tag="v")
            nc.vector.memset(v_t[:, DIM:], 0.0)
            nc.vector.tensor_scalar(v_t[:, 0:DIM], diff, inv_n, None, ALU.mult)
            scr = small.tile([P, DIM], F32, tag="scr")
            nc.vector.scalar_tensor_tensor(
                out=scr, in0=v_t[:, 0:DIM], scalar=MIN_EIG, in1=v_t[:, 0:DIM],
                op0=ALU.mult, op1=ALU.mult,
                accum_out=vsq_all[:, it:it + 1])

            # transposes: xT (augmented with ones row), diffT
            xT_ps = ps_a.tile([DIM, P], F32, tag="ps1")
            nc.tensor.transpose(xT_ps, x_t, ident_sb)
            xTa = small.tile([DIM + 1, P], F32, tag="xTa")
            nc.scalar.copy(out=xTa[0:DIM, :], in_=xT_ps)
            nc.vector.memset(xTa[DIM:DIM + 1, :], 1.0)
            dT_ps = ps_a.tile([DIM, P], F32, tag="ps1")
            nc.tensor.transpose(dT_ps, diff, ident_sb)
            dT = small.tile([DIM, P], F32, tag="dT")
            nc.scalar.copy(out=dT, in_=dT_ps)

            a_ps = ps_a.tile([P, HID], F32, tag="ps1")
            nc.tensor.matmul(a_ps, xTa, w1aug_sb, start=True, stop=True)
            a_sb = small.tile([P, HID], F32, tag="a")
            nc.scalar.copy(out=a_sb, in_=a_ps)
            d_ps = ps_a.tile([P, HID], F32, tag="ps1")
            nc.tensor.matmul(d_ps, dT, w1aug_sb[0:DIM, :], start=True, stop=True)
            d_sb = small.tile([P, HID], F32, tag="d")
            nc.scalar.copy(out=d_sb, in_=d_ps)

            if b2_nonzero:
                vT_ps = ps_a.tile([DIM, P], F32, tag="ps1")
                nc.tensor.transpose(vT_ps, v_t[:, 0:DIM], ident_sb)
                vT = small.tile([DIM, P], F32, tag="vT")
                nc.scalar.copy(out=vT, in_=vT_ps)
                zb_ps = ps_a.tile([P, DIM], F32, tag="ps1")
                nc.tensor.matmul(zb_ps, vT, cb_sb, start=True, stop=True)
                zb = small.tile([P, DIM], F32, tag="zb")
                nc.scalar.copy(out=zb, in_=zb_ps)

            for t in range(n_steps):
                tf = t * inv_n
                hpre = small.tile([P, HID], F32, tag="hpre")
                nc.vector.scalar_tensor_tensor(
                    out=hpre, in0=d_sb, scalar=tf, in1=a_sb,
                    op0=ALU.mult, op1=ALU.add)
                hT_ps = ps_h.tile([HID, P], F32, tag="hT")
                nc.tensor.transpose(hT_ps, hpre, ident_sb)
                hT = small.tile([HID, P], F32, tag="hT")
                nc.scalar.activation(out=hT, in_=hT_ps, func=AF.Relu)
                if MM_DT == F32:
                    hT_mm = hT[:]
                else:
                    hT_r = small.tile([HID, P], MM_DT, tag="hTr")
                    nc.scalar.copy(out=hT_r, in_=hT)
                    hT_mm = hT_r[:]

                u_t = upool.tile([P, GRID], F32, tag="u")
                voffs = [(0, 1), (16, 1), (32, 1), (48, 1), (32, 0), (32, 0)]
                for c in range(6):
                    cp = ps_c.tile([P, 512], F32, tag="comp")
                    nc.tensor.matmul(
                        cp, hT_mm,
                        w2p_sb[:, 512 * c:512 * (c + 1)],
                        start=True, stop=True)
                    off, jstep = voffs[c]
                    u_view = u_t[:, 512 * c:512 * (c + 1)].rearrange(
                        "p (a b) -> p a b", a=16)
                    cp_view = cp[:].rearrange("p (a b) -> p a b", a=16)
                    nc.vector.tensor_tensor(
                        u_view, cp_view, vview(v_t[:], off, jstep), ALU.mult)

                # diagonal comps (full precision fp32 matmul, N=64)
                dr_ps = ps_d.tile([P, DIM], F32, tag="dr")
                nc.tensor.matmul(dr_ps, hT, w2d_sb, start=True, stop=True)
                dr = small.tile([P, DIM], F32, tag="drs")
                nc.scalar.copy(out=dr, in_=dr_ps)

                # z = per-column sums: r[0:64] = TL|BR, r[64:96] = BL
                r_t = work.tile([P, 96], F32, tag="r")
                nc.vector.tensor_reduce(
                    out=r_t, in_=u_t[:].rearrange("p (a b) -> p a b", a=96),
                    axis=mybir.AxisListType.X, op=ALU.add)
                z_t = r_t[:, 0:DIM]
                nc.vector.tensor_tensor(
                    r_t[:, 0:32], r_t[:, 0:32], r_t[:, DIM:96], ALU.add)

                # diag correction: z += (softplus(dr)+0.1-dr)*v, with
                # softplus(x) = relu(x) + ln(1+exp(-|x|))  (no softplus table)
                # so softplus(x)+0.1-x = relu(-x) + ln(1+exp(-|x|)) + 0.1
                if b2_nonzero:
                    nc.vector.tensor_tensor(dr, dr, b2d_sb, ALU.add)
                sp = small.tile([P, DIM], F32, tag="sp")
                nc.scalar.activation(out=sp, in_=dr, func=AF.Abs)
                nc.scalar.activation(out=sp, in_=sp, func=AF.Exp, scale=-1.0)
                nc.scalar.activation(out=sp, in_=sp, func=AF.Ln, bias=1.0)
                rn = small.tile([P, DIM], F32, tag="rn")
                nc.scalar.activation(out=rn, in_=dr, func=AF.Relu, scale=-1.0)
                nc.vector.scalar_tensor_tensor(
                    out=sp, in0=rn, scalar=0.1, in1=sp,
                    op0=ALU.add, op1=ALU.add)
                nc.vector.tensor_tensor(sp, sp, v_t[:, 0:DIM], ALU.mult)
                nc.vector.tensor_tensor(z_t, z_t, sp, ALU.add)
                if b2_nonzero:
                    nc.vector.tensor_tensor(z_t, z_t, zb, ALU.add)

                zscr = small.tile([P, DIM], F32, tag="zscr")
                nc.vector.scalar_tensor_tensor(
                    out=zscr, in0=z_t, scalar=1.0, in1=z_t,
                    op0=ALU.mult, op1=ALU.mult,
                    accum_out=inner_all[:, it, t:t + 1])

        # tail: totals (sqrt table loaded once, after all softplus work)
        for it in range(N_TILES):
            r0 = it * P
            ia2 = small.tile([P, n_steps], F32, tag="ia2")
            nc.vector.scalar_tensor_tensor(
                out=ia2, in0=inner_all[:, it, :],
                scalar=vsq_all[:, it:it + 1],
                in1=thr_sb.broadcast_to([P, n_steps]),
                op0=ALU.add, op1=ALU.max)
            ds = small.tile([P, n_steps], F32, tag="ds")
            nc.scalar.activation(out=ds, in_=ia2, func=AF.Sqrt)
            tot = small.tile([P, 1], F32, tag="tot")
            nc.vector.tensor_reduce(out=tot, in_=ds,
                                    axis=mybir.AxisListType.X, op=ALU.add)
            nc.sync.dma_start(out=outt.ap()[r0:r0 + P, :], in_=tot)

    nc.compile()
    return nc


_PROG_CACHE = {}


def kernel(x, y, W1, b1, W2, b2, n_steps):
    x = np.asarray(x, np.float32)
    y = np.asarray(y, np.float32)
    W1 = np.asarray(W1, np.float32)
    b1 = np.asarray(b1, np.float32)
    W2 = np.asarray(W2, np.float32)
    b2 = np.asarray(b2, np.float32)
    n = int(n_steps)
    b2nz = bool(np.any(b2))

    key = (n, b2nz)
    if key not in _PROG_CACHE:
        _PROG_CACHE[key] = _build_program(n, b2nz)
    nc = _PROG_CACHE[key]

    w1aug = np.concatenate([W1, b1[None, :]], axis=0).astype(np.float32)
    w2p = _build_w2p(W2)
    diag_cols = np.array([_tri_idx(j, j) for j in range(DIM)])
    w2d = np.ascontiguousarray(W2[:, diag_cols])
    identm = np.eye(P, dtype=np.float32)
    cbm = np.zeros((DIM, DIM), np.float32)
    if b2nz:
        for j in range(DIM):
            for i in range(j + 1, DIM):
                cbm[i, j] = b2[_tri_idx(i, j)]
    b2dm = b2[diag_cols][None, :].astype(np.float32)

    in_maps = []
    for c in range(N_CORES):
        sl = slice(c * B_CORE, (c + 1) * B_CORE)
        in_maps.append({
            "xin": np.ascontiguousarray(x[sl]),
            "yin": np.ascontiguousarray(y[sl]),
            "w1aug": w1aug, "w2p": w2p, "w2d": w2d,
            "ident": identm, "cb": cbm, "b2d": b2dm,
        })

    res = run_bass_kernel_spmd(nc, in_maps, core_ids=list(range(N_CORES)))
    out = np.concatenate([res.results[c]["outt"][:, 0] for c in range(N_CORES)])
    return out.astype(np.float32)
